# revision 2
# baseline (speedup 1.0000x reference)
"""Trainium2 Bass kernel for a bilinear critic:

    xe = relu(x @ Wx1 + bx1) @ Wx2 + bx2          # [B, 32]
    ye = relu(y @ Wy1 + by1) @ Wy2 + by2          # [B, 32]
    scores = (xe @ W_bil) @ ye.T + b_bil[0]       # [B, B]
    returns (scores, 0.0)

B=8192, D_IN=64, D_HID=256, D_EMB=32. 8 NeuronCores, data-parallel over
rows of x (1024 rows/core); y embedding computed redundantly on every
core (the MLP is tiny next to the 256 MiB output write, which is the
roofline term). Everything on-chip is kept in TRANSPOSED layout
([feature, batch]) so the contraction dim lands on SBUF partitions with
no on-chip transposes; the host feeds x.T / y.T.

b_bil is folded into the final GEMM by augmenting the contraction dim:
zi = xe @ W_bil gets an extra row of b_bil, ye gets an extra row of
ones, so scores = zi_aug.T @ ye_aug includes the +b_bil term.
"""

import numpy as np
from contextlib import ExitStack

import concourse.bass as bass
import concourse.bacc as bacc
import concourse.tile as tile
from concourse import mybir
from concourse.bass_utils import run_bass_kernel_spmd

B, D_IN, D_HID, D_EMB = 8192, 64, 256, 32
N_CORES = 8
R = B // N_CORES            # 1024 rows of x per core
CH = 512                    # column-chunk width (one PSUM bank of fp32)
NCH = B // CH               # 16 chunks over y's batch dim
RCH = R // CH               # 2 chunks over the core's x rows
F32 = mybir.dt.float32

Relu = mybir.ActivationFunctionType.Relu
Ident = mybir.ActivationFunctionType.Identity

TRACE = False               # test.py flips this to profile
LAST_RESULT = None          # BassKernelResults of the last run

_NC = None


def _emit(ctx, tc, t):
    nc = tc.nc
    PS = bass.MemorySpace.PSUM

    consts = ctx.enter_context(tc.tile_pool(name="consts", bufs=1))
    ypool = ctx.enter_context(tc.tile_pool(name="ycols", bufs=3))
    hpool = ctx.enter_context(tc.tile_pool(name="h", bufs=2))
    yepool = ctx.enter_context(tc.tile_pool(name="ye", bufs=3))
    embp = ctx.enter_context(tc.tile_pool(name="emb", bufs=1))
    outp = ctx.enter_context(tc.tile_pool(name="outs", bufs=4))
    ps_h = ctx.enter_context(tc.tile_pool(name="ps_h", bufs=2, space=PS))
    ps_e = ctx.enter_context(tc.tile_pool(name="ps_e", bufs=2, space=PS))
    ps_s = ctx.enter_context(tc.tile_pool(name="ps_s", bufs=4, space=PS))

    def load_const(tag, shape, src):
        sb = consts.tile(shape, F32, tag=tag)
        nc.sync.dma_start(sb[:], src)
        return sb

    wx1_sb = load_const("wx1", [D_IN, D_HID], t["wx1"][:])
    wy1_sb = load_const("wy1", [D_IN, D_HID], t["wy1"][:])
    wx2a = load_const("wx2a", [128, D_EMB], t["wx2"][0:128, :])
    wx2b = load_const("wx2b", [128, D_EMB], t["wx2"][128:256, :])
    wy2a = load_const("wy2a", [128, D_EMB], t["wy2"][0:128, :])
    wy2b = load_const("wy2b", [128, D_EMB], t["wy2"][128:256, :])
    wb_sb = load_const("wbil", [D_EMB, D_EMB], t["wbil"][:])
    bx1a = load_const("bx1a", [128, 1], t["bx1"][0:128, :])
    bx1b = load_const("bx1b", [128, 1], t["bx1"][128:256, :])
    by1a = load_const("by1a", [128, 1], t["by1"][0:128, :])
    by1b = load_const("by1b", [128, 1], t["by1"][128:256, :])
    bx2_sb = load_const("bx2", [D_EMB, 1], t["bx2"][:])
    by2_sb = load_const("by2", [D_EMB, 1], t["by2"][:])
    bb_sb = load_const("bbil", [1, 1], t["bbil"][:])
    xt_sb = load_const("xt", [D_IN, R], t["xt"][:])

    zi_sb = embp.tile([D_EMB + 1, R], F32, tag="zi")
    xe_sb = embp.tile([D_EMB, R], F32, tag="xe")
    zrow = consts.tile([1, R], F32, tag="zrow")
    nc.gpsimd.memset(zrow[:], 0.0)

    # ---- x embedding: hx = relu(Wx1.T @ x.T + bx1); xe = Wx2.T @ hx + bx2
    hx_tiles = []
    for mb in range(2):
        hx = embp.tile([128, R], F32, tag=f"hx{mb}")
        for nb in range(RCH):
            cs = slice(nb * CH, (nb + 1) * CH)
            ph = ps_h.tile([128, CH], F32, tag="ph")
            nc.tensor.matmul(
                ph[:], wx1_sb[:, mb * 128:(mb + 1) * 128], xt_sb[:, cs],
                start=True, stop=True,
            )
            nc.scalar.activation(
                hx[:, cs], ph[:], Relu, bias=(bx1a if mb == 0 else bx1b)[:]
            )
        hx_tiles.append(hx)

    for nb in range(RCH):
        cs = slice(nb * CH, (nb + 1) * CH)
        pe_ = ps_e.tile([D_EMB, CH], F32, tag="pe")
        nc.tensor.matmul(pe_[:], wx2a[:], hx_tiles[0][:, cs], start=True, stop=False)
        nc.tensor.matmul(pe_[:], wx2b[:], hx_tiles[1][:, cs], start=False, stop=True)
        nc.scalar.activation(xe_sb[:, cs], pe_[:], Ident, bias=bx2_sb[:])

    # ---- zi = W_bil.T @ xe (transposed view: zi.T = W_bil.T @ xe.T)
    for nb in range(RCH):
        cs = slice(nb * CH, (nb + 1) * CH)
        pz = ps_e.tile([D_EMB, CH], F32, tag="pe")
        nc.tensor.matmul(pz[:], wb_sb[:], xe_sb[:, cs], start=True, stop=True)
        nc.vector.tensor_copy(zi_sb[0:D_EMB, cs], pz[:])
    # bias row: zi_aug[32, :] = b_bil
    nc.scalar.activation(zi_sb[D_EMB:D_EMB + 1, :], zrow[:], Ident, bias=bb_sb[:])

    # ---- y embedding + scores, interleaved per 512-wide column chunk
    for nb in range(NCH):
        cs = slice(nb * CH, (nb + 1) * CH)
        ycol = ypool.tile([D_IN, CH], F32, tag="ycol")
        nc.sync.dma_start(ycol[:], t["yt"][:, cs])

        hy_tiles = []
        for mb in range(2):
            ph = ps_h.tile([128, CH], F32, tag="ph")
            nc.tensor.matmul(
                ph[:], wy1_sb[:, mb * 128:(mb + 1) * 128], ycol[:],
                start=True, stop=True,
            )
            hy = hpool.tile([128, CH], F32, tag=f"hy{mb}")
            nc.scalar.activation(
                hy[:], ph[:], Relu, bias=(by1a if mb == 0 else by1b)[:]
            )
            hy_tiles.append(hy)

        pey = ps_e.tile([D_EMB, CH], F32, tag="pe")
        nc.tensor.matmul(pey[:], wy2a[:], hy_tiles[0][:], start=True, stop=False)
        nc.tensor.matmul(pey[:], wy2b[:], hy_tiles[1][:], start=False, stop=True)

        ye = yepool.tile([D_EMB + 1, CH], F32, tag="ye")
        nc.scalar.activation(ye[0:D_EMB, :], pey[:], Ident, bias=by2_sb[:])
        nc.gpsimd.memset(ye[D_EMB:D_EMB + 1, :], 1.0)

        for mb in range(N_CORES):  # 8 row-blocks of 128 within this core's R rows
            ps = ps_s.tile([128, CH], F32, tag="ps")
            nc.tensor.matmul(
                ps[:], zi_sb[:, mb * 128:(mb + 1) * 128], ye[:],
                start=True, stop=True,
            )
            ot = outp.tile([128, CH], F32, tag="ot")
            if mb % 2 == 0:
                nc.vector.tensor_copy(ot[:], ps[:])
            else:
                nc.scalar.activation(ot[:], ps[:], Ident)
            nc.sync.dma_start(t["scores"][mb * 128:(mb + 1) * 128, cs], ot[:])


def _build():
    nc = bacc.Bacc(
        "TRN2", target_bir_lowering=False, debug=False, num_devices=N_CORES
    )
    t = {}
    def din(name, shape):
        t[name] = nc.dram_tensor(name, shape, F32, kind="ExternalInput").ap()
    din("xt", [D_IN, R])
    din("yt", [D_IN, B])
    din("wx1", [D_IN, D_HID])
    din("bx1", [D_HID, 1])
    din("wx2", [D_HID, D_EMB])
    din("bx2", [D_EMB, 1])
    din("wy1", [D_IN, D_HID])
    din("by1", [D_HID, 1])
    din("wy2", [D_HID, D_EMB])
    din("by2", [D_EMB, 1])
    din("wbil", [D_EMB, D_EMB])
    din("bbil", [1, 1])
    t["scores"] = nc.dram_tensor("scores", [R, B], F32, kind="ExternalOutput").ap()

    with tile.TileContext(nc) as tc:
        with ExitStack() as ctx:
            _emit(ctx, tc, t)
    nc.compile()
    return nc


def kernel(**inputs):
    global _NC, LAST_RESULT
    f = lambda k: np.ascontiguousarray(np.asarray(inputs[k], dtype=np.float32))
    xt_full = np.ascontiguousarray(f("x").T)    # [64, 8192]
    yt_full = np.ascontiguousarray(f("y").T)    # [64, 8192]

    base = {
        "yt": yt_full,
        "wx1": f("Wx1"),
        "bx1": f("bx1").reshape(D_HID, 1),
        "wx2": f("Wx2"),
        "bx2": f("bx2").reshape(D_EMB, 1),
        "wy1": f("Wy1"),
        "by1": f("by1").reshape(D_HID, 1),
        "wy2": f("Wy2"),
        "by2": f("by2").reshape(D_EMB, 1),
        "wbil": f("W_bil"),
        "bbil": f("b_bil").reshape(1, 1),
    }
    in_maps = [
        {**base, "xt": np.ascontiguousarray(xt_full[:, c * R:(c + 1) * R])}
        for c in range(N_CORES)
    ]

    if _NC is None:
        _NC = _build()
    res = run_bass_kernel_spmd(_NC, in_maps, list(range(N_CORES)), trace=TRACE)
    LAST_RESULT = res
    scores = np.concatenate(
        [res.results[c]["scores"] for c in range(N_CORES)], axis=0
    )
    return scores, np.zeros((), np.float32)


# revision 5
# speedup vs baseline: 1.5429x; 1.5429x over previous
"""Trainium2 Bass kernel for a bilinear critic:

    xe = relu(x @ Wx1 + bx1) @ Wx2 + bx2          # [B, 32]
    ye = relu(y @ Wy1 + by1) @ Wy2 + by2          # [B, 32]
    scores = (xe @ W_bil) @ ye.T + b_bil[0]       # [B, B]
    returns (scores, 0.0)

B=8192, D_IN=64, D_HID=256, D_EMB=32. 8 NeuronCores, data-parallel over
rows of x (1024 rows/core); the y embedding is computed redundantly on
every core (the MLP is tiny next to the 256 MiB output write, which is
the roofline term). Everything on-chip is kept in TRANSPOSED layout
([feature, batch]) so the contraction dim lands on SBUF partitions with
no on-chip transposes; the host feeds x.T / y.T.

fp32 matmuls run at 1/4 PE rate, so all big GEMMs use an exact 3-term
bf16 hi/lo split (a = ah + al; a@b ~ ah@bh + al@bh + ah@bl, error
~2^-18): the PE streams bf16 at full rate and accumulates in fp32 PSUM.
Splits of inputs/weights are host-precomputed; h's split is computed
on-chip (ACT relu -> hh, DVE (max(psum,0) - hh) -> hl).

Bias folding:
  - by1 enters the layer-1 matmul as two extra contraction rows
    (bf16 hi/lo) against ones-rows of the input stack (exact, and keeps
    relu after the full affine).
  - W_bil and by2 are folded on host into the y-side layer-2:
    ywe = ye @ W_bil^T, so scores = xe_aug^T @ ywe_aug.
  - b_bil enters the scores matmul as two extra hi/lo rows against
    ones-rows on the y side.
"""

import numpy as np
from contextlib import ExitStack

import ml_dtypes
import concourse.bass as bass
import concourse.bacc as bacc
import concourse.tile as tile
from concourse import mybir
from concourse.bass_utils import run_bass_kernel_spmd

B, D_IN, D_HID, D_EMB = 8192, 64, 256, 32
N_CORES = 8
R = B // N_CORES            # 1024 rows of x per core
CH = 512                    # column-chunk width (one PSUM bank of fp32)
NCH = B // CH               # 16 chunks over y's batch dim
RCH = R // CH               # 2 chunks over the core's x rows
K1 = 3 * D_IN + 2           # 194: layer-1 stacked contraction
KS = 3 * D_EMB + 2          # 98: scores stacked contraction
F32 = mybir.dt.float32
BF16 = mybir.dt.bfloat16
NPBF = ml_dtypes.bfloat16

Relu = mybir.ActivationFunctionType.Relu
Ident = mybir.ActivationFunctionType.Identity
ADD = mybir.AluOpType.add
SUB = mybir.AluOpType.subtract
MAX = mybir.AluOpType.max

TRACE = False               # test.py flips this to profile
LAST_RESULT = None          # BassKernelResults of the last run

_NC = None


def _emit(ctx, tc, t):
    nc = tc.nc
    PS = bass.MemorySpace.PSUM

    consts = ctx.enter_context(tc.tile_pool(name="consts", bufs=1))
    ypool = ctx.enter_context(tc.tile_pool(name="ycols", bufs=3))
    hpool = ctx.enter_context(tc.tile_pool(name="h", bufs=2))
    yepool = ctx.enter_context(tc.tile_pool(name="ye", bufs=3))
    embp = ctx.enter_context(tc.tile_pool(name="emb", bufs=1))
    outp = ctx.enter_context(tc.tile_pool(name="outs", bufs=4))
    ps_h = ctx.enter_context(tc.tile_pool(name="ps_h", bufs=2, space=PS))
    ps_e = ctx.enter_context(tc.tile_pool(name="ps_e", bufs=2, space=PS))
    ps_s = ctx.enter_context(tc.tile_pool(name="ps_s", bufs=4, space=PS))

    def load_const(tag, shape, dt, src):
        sb = consts.tile(shape, dt, tag=tag)
        nc.sync.dma_start(sb[:], src)
        return sb

    # layer-1 stacked weights [K1, 256]: [w1h; w1l; w1h; b1h; b1l]
    w1x_a = load_const("w1x_a", [128, D_HID], BF16, t["wx1st"][0:128, :])
    w1x_b = load_const("w1x_b", [K1 - 128, D_HID], BF16, t["wx1st"][128:K1, :])
    w1y_a = load_const("w1y_a", [128, D_HID], BF16, t["wy1st"][0:128, :])
    w1y_b = load_const("w1y_b", [K1 - 128, D_HID], BF16, t["wy1st"][128:K1, :])
    # layer-2 stacked weights [512, 32]: [w2h(256); w2l(256)] as 4 K-chunks
    w2x = [load_const(f"w2x{i}", [128, D_EMB], BF16,
                      t["wx2st"][i * 128:(i + 1) * 128, :]) for i in range(4)]
    w2y = [load_const(f"w2y{i}", [128, D_EMB], BF16,
                      t["wy2st"][i * 128:(i + 1) * 128, :]) for i in range(4)]
    bx2_sb = load_const("bx2", [D_EMB, 1], F32, t["bx2"][:])
    by2_sb = load_const("by2", [D_EMB, 1], F32, t["by2f"][:])
    bb_sb = load_const("bbil", [2, 1], F32, t["bbst"][:])
    # x input stack [K1, R]: [xh; xh; xl; 1; 1]
    x_a = load_const("x_a", [128, R], BF16, t["xts"][0:128, :])
    x_b = load_const("x_b", [K1 - 128, R], BF16, t["xts"][128:K1, :])

    xst = embp.tile([KS, R], BF16, tag="xst")   # [xh; xl; xh; bh; bl]
    zero2 = consts.tile([2, R], BF16, tag="zero2")
    nc.gpsimd.memset(zero2[:], 0.0)

    def l2_matmuls(pe_, w2, hh, hl, cs=None):
        """ye[32, ch] += w2h.hh + w2l.hh + w2h.hl (6 accumulating mms)."""
        pairs = [(w2[0], hh[0]), (w2[1], hh[1]), (w2[2], hh[0]),
                 (w2[3], hh[1]), (w2[0], hl[0]), (w2[1], hl[1])]
        for i, (w, h) in enumerate(pairs):
            rhs = h[:] if cs is None else h[:, cs]
            nc.tensor.matmul(pe_[:], w[:], rhs,
                             start=(i == 0), stop=(i == len(pairs) - 1))

    # ---- x embedding (8 row-blocks of this core's 1024 rows)
    hhx = [[None, None] for _ in range(RCH)]
    hlx = [[None, None] for _ in range(RCH)]
    for nb in range(RCH):
        cs = slice(nb * CH, (nb + 1) * CH)
        for mb in range(2):
            msl = slice(mb * 128, (mb + 1) * 128)
            ph = ps_h.tile([128, CH], F32, tag="ph")
            nc.tensor.matmul(ph[:], w1x_a[:, msl], x_a[:, cs], start=True, stop=False)
            nc.tensor.matmul(ph[:], w1x_b[:, msl], x_b[:, cs], start=False, stop=True)
            hh = embp.tile([128, CH], BF16, tag=f"hhx{nb}{mb}")
            nc.scalar.activation(hh[:], ph[:], Relu)
            hl = embp.tile([128, CH], BF16, tag=f"hlx{nb}{mb}")
            nc.vector.scalar_tensor_tensor(hl[:], ph[:], 0.0, hh[:], MAX, SUB)
            hhx[nb][mb] = hh
            hlx[nb][mb] = hl

    for nb in range(RCH):
        cs = slice(nb * CH, (nb + 1) * CH)
        pex = ps_e.tile([D_EMB, CH], F32, tag="pe")
        l2_matmuls(pex, w2x, hhx[nb], hlx[nb])
        nc.scalar.activation(xst[0:32, cs], pex[:], Ident, bias=bx2_sb[:])
        nc.vector.scalar_tensor_tensor(
            xst[32:64, cs], pex[:], bx2_sb[:], xst[0:32, cs], ADD, SUB)
        nc.vector.tensor_copy(xst[64:96, cs], xst[0:32, cs])
    # scores-bias rows: xst[96] = bf16_hi(b_bil), xst[97] = bf16_lo(b_bil)
    nc.scalar.activation(xst[96:98, :], zero2[:], Ident, bias=bb_sb[:])

    # ---- y embedding + scores, interleaved per 512-wide column chunk
    for nb in range(NCH):
        cs = slice(nb * CH, (nb + 1) * CH)
        y_a = ypool.tile([128, CH], BF16, tag="y_a")
        nc.sync.dma_start(y_a[:], t["yts"][0:128, cs])
        y_b = ypool.tile([K1 - 128, CH], BF16, tag="y_b")
        nc.sync.dma_start(y_b[:], t["yts"][128:K1, cs])

        hhy, hly = [], []
        for mb in range(2):
            msl = slice(mb * 128, (mb + 1) * 128)
            ph = ps_h.tile([128, CH], F32, tag="ph")
            nc.tensor.matmul(ph[:], w1y_a[:, msl], y_a[:], start=True, stop=False)
            nc.tensor.matmul(ph[:], w1y_b[:, msl], y_b[:], start=False, stop=True)
            hh = hpool.tile([128, CH], BF16, tag=f"hhy{mb}")
            nc.scalar.activation(hh[:], ph[:], Relu)
            hl = hpool.tile([128, CH], BF16, tag=f"hly{mb}")
            nc.vector.scalar_tensor_tensor(hl[:], ph[:], 0.0, hh[:], MAX, SUB)
            hhy.append(hh)
            hly.append(hl)

        pey = ps_e.tile([D_EMB, CH], F32, tag="pe")
        l2_matmuls(pey, w2y, hhy, hly)

        # xst is [xh; xl; xh; bh; bl], so yst must pair as [yh; yh; yl; 1; 1]
        yst = yepool.tile([KS, CH], BF16, tag="yst")
        nc.scalar.activation(yst[0:32, :], pey[:], Ident, bias=by2_sb[:])
        nc.vector.tensor_copy(yst[32:64, :], yst[0:32, :])
        nc.vector.scalar_tensor_tensor(
            yst[64:96, :], pey[:], by2_sb[:], yst[0:32, :], ADD, SUB)
        nc.gpsimd.memset(yst[96:98, :], 1.0)

        for mb in range(8):  # 8 row-blocks of 128 within this core's R rows
            ps = ps_s.tile([128, CH], F32, tag="ps")
            nc.tensor.matmul(
                ps[:], xst[:, mb * 128:(mb + 1) * 128], yst[:],
                start=True, stop=True,
            )
            ot = outp.tile([128, CH], F32, tag="ot")
            if mb % 2 == 0:
                nc.vector.tensor_copy(ot[:], ps[:])
            else:
                nc.scalar.activation(ot[:], ps[:], Ident)
            nc.sync.dma_start(t["scores"][mb * 128:(mb + 1) * 128, cs], ot[:])


def _build():
    nc = bacc.Bacc(
        "TRN2", target_bir_lowering=False, debug=False, num_devices=N_CORES
    )
    t = {}

    def din(name, shape, dt):
        t[name] = nc.dram_tensor(name, shape, dt, kind="ExternalInput").ap()

    din("xts", [K1, R], BF16)
    din("yts", [K1, B], BF16)
    din("wx1st", [K1, D_HID], BF16)
    din("wy1st", [K1, D_HID], BF16)
    din("wx2st", [512, D_EMB], BF16)
    din("wy2st", [512, D_EMB], BF16)
    din("bx2", [D_EMB, 1], F32)
    din("by2f", [D_EMB, 1], F32)
    din("bbst", [2, 1], F32)
    t["scores"] = nc.dram_tensor("scores", [R, B], F32, kind="ExternalOutput").ap()

    with tile.TileContext(nc) as tc:
        with ExitStack() as ctx:
            _emit(ctx, tc, t)
    nc.compile()
    return nc


def _split(a):
    """f32 array -> (hi, lo) bf16 arrays with a ~= hi + lo (err ~2^-18)."""
    hi = a.astype(NPBF)
    lo = (a - hi.astype(np.float32)).astype(NPBF)
    return hi, lo


def _stack1(w1, b1):
    """Layer-1 lhsT stack [K1, 256]: [w1h; w1l; w1h; b1h; b1l]."""
    wh, wl = _split(w1)                      # [64, 256]
    bh, bl = _split(b1.reshape(1, -1))       # [1, 256]
    return np.concatenate([wh, wl, wh, bh, bl], axis=0)


def _stackin(aT):
    """Layer-1 rhs stack [K1, n]: [ah; ah; al; 1; 1]."""
    ah, al = _split(aT)                      # [64, n]
    ones = np.ones((2, aT.shape[1]), NPBF)
    return np.concatenate([ah, ah, al, ones], axis=0)


def kernel(**inputs):
    global _NC, LAST_RESULT
    f = lambda k: np.ascontiguousarray(np.asarray(inputs[k], dtype=np.float32))

    x, y = f("x"), f("y")
    xts_full = _stackin(np.ascontiguousarray(x.T))   # [194, 8192]
    yts = _stackin(np.ascontiguousarray(y.T))        # [194, 8192]

    # fold W_bil (and by2) into the y-side layer 2: ywe = ye @ W_bil^T
    Wb = f("W_bil").astype(np.float64)
    wy2f = (f("Wy2").astype(np.float64) @ Wb.T).astype(np.float32)
    by2f = (f("by2").astype(np.float64) @ Wb.T).astype(np.float32)

    def stack2(w2):
        wh, wl = _split(w2)                  # [256, 32]
        return np.concatenate([wh, wl], axis=0)

    bh, bl = _split(f("b_bil").reshape(1, 1))
    base = {
        "yts": yts,
        "wx1st": _stack1(f("Wx1"), f("bx1")),
        "wy1st": _stack1(f("Wy1"), f("by1")),
        "wx2st": stack2(f("Wx2")),
        "wy2st": stack2(wy2f),
        "bx2": f("bx2").reshape(D_EMB, 1),
        "by2f": by2f.reshape(D_EMB, 1),
        "bbst": np.array(
            [[np.float32(bh[0, 0])], [np.float32(bl[0, 0])]], np.float32),
    }
    in_maps = [
        {**base, "xts": np.ascontiguousarray(xts_full[:, c * R:(c + 1) * R])}
        for c in range(N_CORES)
    ]

    if _NC is None:
        _NC = _build()
    res = run_bass_kernel_spmd(_NC, in_maps, list(range(N_CORES)), trace=TRACE)
    LAST_RESULT = res
    scores = np.concatenate(
        [res.results[c]["scores"] for c in range(N_CORES)], axis=0
    )
    return scores, np.zeros((), np.float32)


# revision 9
# speedup vs baseline: 2.0351x; 1.3190x over previous
"""Trainium2 Bass kernel for a bilinear critic:

    xe = relu(x @ Wx1 + bx1) @ Wx2 + bx2          # [B, 32]
    ye = relu(y @ Wy1 + by1) @ Wy2 + by2          # [B, 32]
    scores = (xe @ W_bil) @ ye.T + b_bil[0]       # [B, B]
    returns (scores, 0.0)

B=8192, D_IN=64, D_HID=256, D_EMB=32. 8 NeuronCores, data-parallel over
rows of x (1024 rows/core); the y embedding is computed redundantly on
every core (the MLP is tiny next to the 256 MiB output write, which is
the roofline term). Everything on-chip is kept in TRANSPOSED layout
([feature, batch]) so the contraction dim lands on SBUF partitions with
no on-chip transposes; the host feeds x.T / y.T.

fp32 matmuls run at 1/4 PE rate, so all big GEMMs use an exact 3-term
bf16 hi/lo split (a = ah + al; a@b ~ ah@bh + al@bh + ah@bl, error
~2^-18): the PE streams bf16 at full rate and accumulates in fp32 PSUM.
Splits of inputs/weights are host-precomputed; h's split is computed
on-chip (ACT relu -> hh, DVE (max(psum,0) - hh) -> hl).

Bias folding:
  - by1 enters the layer-1 matmul as two extra contraction rows
    (bf16 hi/lo) against ones-rows of the input stack (exact, and keeps
    relu after the full affine).
  - W_bil and by2 are folded on host into the y-side layer-2:
    ywe = ye @ W_bil^T, so scores = xe_aug^T @ ywe_aug.
  - b_bil enters the scores matmul as two extra hi/lo rows against
    ones-rows on the y side.
"""

import numpy as np
from contextlib import ExitStack

import ml_dtypes
import concourse.bass as bass
import concourse.bacc as bacc
import concourse.tile as tile
from concourse import mybir
from concourse.bass_utils import run_bass_kernel_spmd

B, D_IN, D_HID, D_EMB = 8192, 64, 256, 32
N_CORES = 8
R = B // N_CORES            # 1024 rows of x per core
CH = 512                    # column-chunk width (one PSUM bank of fp32)
NCH = B // CH               # 16 chunks over y's batch dim
RCH = R // CH               # 2 chunks over the core's x rows
K1 = 3 * D_IN + 2           # 194: layer-1 stacked contraction
KS = 3 * D_EMB + 2          # 98: scores stacked contraction
F32 = mybir.dt.float32
BF16 = mybir.dt.bfloat16
NPBF = ml_dtypes.bfloat16

Relu = mybir.ActivationFunctionType.Relu
Ident = mybir.ActivationFunctionType.Identity
ADD = mybir.AluOpType.add
SUB = mybir.AluOpType.subtract
MAX = mybir.AluOpType.max

TRACE = False               # test.py flips this to profile
LAST_RESULT = None          # BassKernelResults of the last run

_NC = None


def _emit(ctx, tc, t):
    nc = tc.nc
    PS = bass.MemorySpace.PSUM

    consts = ctx.enter_context(tc.tile_pool(name="consts", bufs=1))
    ypool = ctx.enter_context(tc.tile_pool(name="ycols", bufs=4))
    hpool = ctx.enter_context(tc.tile_pool(name="h", bufs=3))
    yepool = ctx.enter_context(tc.tile_pool(name="ye", bufs=4))
    embp = ctx.enter_context(tc.tile_pool(name="emb", bufs=1))
    outp = ctx.enter_context(tc.tile_pool(name="outs", bufs=6))
    ps_h = ctx.enter_context(tc.tile_pool(name="ps_h", bufs=2, space=PS))
    ps_e = ctx.enter_context(tc.tile_pool(name="ps_e", bufs=2, space=PS))
    ps_s = ctx.enter_context(tc.tile_pool(name="ps_s", bufs=2, space=PS))

    def load_const(tag, shape, dt, src):
        sb = consts.tile(shape, dt, tag=tag)
        nc.gpsimd.dma_start(sb[:], src)
        return sb

    # layer-1 stacked weights [K1, 256]: [w1h; w1l; w1h; b1h; b1l]
    w1x_a = load_const("w1x_a", [128, D_HID], BF16, t["wx1st"][0:128, :])
    w1x_b = load_const("w1x_b", [K1 - 128, D_HID], BF16, t["wx1st"][128:K1, :])
    w1y_a = load_const("w1y_a", [128, D_HID], BF16, t["wy1st"][0:128, :])
    w1y_b = load_const("w1y_b", [K1 - 128, D_HID], BF16, t["wy1st"][128:K1, :])
    # layer-2 stacked weights [512, 32]: [w2h(256); w2l(256)] as 4 K-chunks
    w2x = [load_const(f"w2x{i}", [128, D_EMB], BF16,
                      t["wx2st"][i * 128:(i + 1) * 128, :]) for i in range(4)]
    w2y = [load_const(f"w2y{i}", [128, D_EMB], BF16,
                      t["wy2st"][i * 128:(i + 1) * 128, :]) for i in range(4)]
    bx2_sb = load_const("bx2", [D_EMB, 1], F32, t["bx2"][:])
    by2_sb = load_const("by2", [D_EMB, 1], F32, t["by2f"][:])
    bb_sb = load_const("bbil", [2, 1], F32, t["bbst"][:])
    # x input stack [K1, R]: [xh; xh; xl; 1; 1]
    x_a = load_const("x_a", [128, R], BF16, t["xts"][0:128, :])
    x_b = load_const("x_b", [K1 - 128, R], BF16, t["xts"][128:K1, :])

    xst = embp.tile([KS, R], BF16, tag="xst")   # [xh; xl; xh; bh; bl]
    zero2 = consts.tile([2, R], BF16, tag="zero2")
    nc.gpsimd.memset(zero2[:], 0.0)

    def l2_matmuls(pe_, w2, hh, hl, cs=None):
        """ye[32, ch] += w2h.hh + w2l.hh + w2h.hl (6 accumulating mms)."""
        pairs = [(w2[0], hh[0]), (w2[1], hh[1]), (w2[2], hh[0]),
                 (w2[3], hh[1]), (w2[0], hl[0]), (w2[1], hl[1])]
        for i, (w, h) in enumerate(pairs):
            rhs = h[:] if cs is None else h[:, cs]
            nc.tensor.matmul(pe_[:], w[:], rhs,
                             start=(i == 0), stop=(i == len(pairs) - 1))

    # ---- x embedding (8 row-blocks of this core's 1024 rows)
    hhx = [[None, None] for _ in range(RCH)]
    hlx = [[None, None] for _ in range(RCH)]
    for nb in range(RCH):
        cs = slice(nb * CH, (nb + 1) * CH)
        for mb in range(2):
            msl = slice(mb * 128, (mb + 1) * 128)
            ph = ps_h.tile([128, CH], F32, tag="ph")
            nc.tensor.matmul(ph[:], w1x_a[:, msl], x_a[:, cs], start=True, stop=False)
            nc.tensor.matmul(ph[:], w1x_b[:, msl], x_b[:, cs], start=False, stop=True)
            hh = embp.tile([128, CH], BF16, tag=f"hhx{nb}{mb}")
            nc.scalar.activation(hh[:], ph[:], Relu)
            hl = embp.tile([128, CH], BF16, tag=f"hlx{nb}{mb}")
            nc.vector.scalar_tensor_tensor(hl[:], ph[:], 0.0, hh[:], MAX, SUB)
            hhx[nb][mb] = hh
            hlx[nb][mb] = hl

    for nb in range(RCH):
        cs = slice(nb * CH, (nb + 1) * CH)
        pex = ps_e.tile([D_EMB, CH], F32, tag="pe")
        l2_matmuls(pex, w2x, hhx[nb], hlx[nb])
        nc.scalar.activation(xst[0:32, cs], pex[:], Ident, bias=bx2_sb[:])
        nc.vector.scalar_tensor_tensor(
            xst[32:64, cs], pex[:], bx2_sb[:], xst[0:32, cs], ADD, SUB)
        nc.vector.tensor_copy(xst[64:96, cs], xst[0:32, cs])
    # scores-bias rows: xst[96] = bf16_hi(b_bil), xst[97] = bf16_lo(b_bil)
    nc.scalar.activation(xst[96:98, :], zero2[:], Ident, bias=bb_sb[:])

    # ---- y embedding + scores, interleaved per 512-wide column chunk
    for nb in range(NCH):
        cs = slice(nb * CH, (nb + 1) * CH)
        # SWDGE (gpsimd) for loads: keeps them off the sync-engine HWDGE
        # FIFO, which the big output stores occupy.
        y_a = ypool.tile([128, CH], BF16, tag="y_a")
        nc.gpsimd.dma_start(y_a[:], t["yts"][0:128, cs])
        y_b = ypool.tile([K1 - 128, CH], BF16, tag="y_b")
        nc.gpsimd.dma_start(y_b[:], t["yts"][128:K1, cs])

        hhy, hly = [], []
        for mb in range(2):
            msl = slice(mb * 128, (mb + 1) * 128)
            ph = ps_h.tile([128, CH], F32, tag="ph")
            nc.tensor.matmul(ph[:], w1y_a[:, msl], y_a[:], start=True, stop=False)
            nc.tensor.matmul(ph[:], w1y_b[:, msl], y_b[:], start=False, stop=True)
            hh = hpool.tile([128, CH], BF16, tag=f"hhy{mb}")
            nc.scalar.activation(hh[:], ph[:], Relu)
            hl = hpool.tile([128, CH], BF16, tag=f"hly{mb}")
            nc.vector.scalar_tensor_tensor(hl[:], ph[:], 0.0, hh[:], MAX, SUB)
            hhy.append(hh)
            hly.append(hl)

        pey = ps_e.tile([D_EMB, CH], F32, tag="pe")
        l2_matmuls(pey, w2y, hhy, hly)

        # xst is [xh; xl; xh; bh; bl], so yst must pair as [yh; yh; yl; 1; 1]
        yst = yepool.tile([KS, CH], BF16, tag="yst")
        nc.scalar.activation(yst[0:32, :], pey[:], Ident, bias=by2_sb[:])
        nc.vector.tensor_copy(yst[32:64, :], yst[0:32, :])
        nc.vector.scalar_tensor_tensor(
            yst[64:96, :], pey[:], by2_sb[:], yst[0:32, :], ADD, SUB)
        nc.gpsimd.memset(yst[96:98, :], 1.0)

        # scores: pairs of row-blocks share a 2-bank psum tile so the
        # PSUM->SBUF copy and the store both move 128x1024 at once
        # (amortized op overhead, 4 KB DMA descriptors).
        for mp in range(4):  # pairs of 128-row blocks -> [256 rows, 512 cols]
            ps = ps_s.tile([128, 2 * CH], F32, tag="ps")
            for half in range(2):
                mb = 2 * mp + half
                nc.tensor.matmul(
                    ps[:, half * CH:(half + 1) * CH],
                    xst[:, mb * 128:(mb + 1) * 128], yst[:],
                    start=True, stop=True,
                )
            ot = outp.tile([128, 2 * CH], F32, tag="ot")
            # 5:3 ACT:DVE balance across each 2-chunk window (DVE carries
            # the hl/yl arithmetic, ACT only the relus).
            if (4 * nb + mp) % 8 in (0, 2, 4, 5, 6):
                nc.scalar.activation(ot[:], ps[:], Ident)
            else:
                nc.vector.tensor_copy(ot[:], ps[:])
            # out rows [256] viewed as [128, 2, 512]: row r of the tile
            # holds scores[mp*256 + r] (cols cs) and scores[mp*256+128+r]
            dst = t["scores"][2 * mp * 128:(2 * mp + 2) * 128, cs]
            nc.sync.dma_start(
                dst.rearrange("(h p) c -> p h c", p=128),
                ot[:].rearrange("p (h c) -> p h c", c=CH),
            )


def _build():
    nc = bacc.Bacc(
        "TRN2", target_bir_lowering=False, debug=False, num_devices=N_CORES
    )
    t = {}

    def din(name, shape, dt):
        t[name] = nc.dram_tensor(name, shape, dt, kind="ExternalInput").ap()

    din("xts", [K1, R], BF16)
    din("yts", [K1, B], BF16)
    din("wx1st", [K1, D_HID], BF16)
    din("wy1st", [K1, D_HID], BF16)
    din("wx2st", [512, D_EMB], BF16)
    din("wy2st", [512, D_EMB], BF16)
    din("bx2", [D_EMB, 1], F32)
    din("by2f", [D_EMB, 1], F32)
    din("bbst", [2, 1], F32)
    t["scores"] = nc.dram_tensor("scores", [R, B], F32, kind="ExternalOutput").ap()

    with tile.TileContext(nc) as tc:
        with ExitStack() as ctx:
            _emit(ctx, tc, t)
    nc.compile()
    return nc


def _split(a):
    """f32 array -> (hi, lo) bf16 arrays with a ~= hi + lo (err ~2^-18)."""
    hi = a.astype(NPBF)
    lo = (a - hi.astype(np.float32)).astype(NPBF)
    return hi, lo


def _stack1(w1, b1):
    """Layer-1 lhsT stack [K1, 256]: [w1h; w1l; w1h; b1h; b1l]."""
    wh, wl = _split(w1)                      # [64, 256]
    bh, bl = _split(b1.reshape(1, -1))       # [1, 256]
    return np.concatenate([wh, wl, wh, bh, bl], axis=0)


def _stackin(aT):
    """Layer-1 rhs stack [K1, n]: [ah; ah; al; 1; 1]."""
    ah, al = _split(aT)                      # [64, n]
    ones = np.ones((2, aT.shape[1]), NPBF)
    return np.concatenate([ah, ah, al, ones], axis=0)


def kernel(**inputs):
    global _NC, LAST_RESULT
    f = lambda k: np.ascontiguousarray(np.asarray(inputs[k], dtype=np.float32))

    x, y = f("x"), f("y")
    xts_full = _stackin(np.ascontiguousarray(x.T))   # [194, 8192]
    yts = _stackin(np.ascontiguousarray(y.T))        # [194, 8192]

    # fold W_bil (and by2) into the y-side layer 2: ywe = ye @ W_bil^T
    Wb = f("W_bil").astype(np.float64)
    wy2f = (f("Wy2").astype(np.float64) @ Wb.T).astype(np.float32)
    by2f = (f("by2").astype(np.float64) @ Wb.T).astype(np.float32)

    def stack2(w2):
        wh, wl = _split(w2)                  # [256, 32]
        return np.concatenate([wh, wl], axis=0)

    bh, bl = _split(f("b_bil").reshape(1, 1))
    base = {
        "yts": yts,
        "wx1st": _stack1(f("Wx1"), f("bx1")),
        "wy1st": _stack1(f("Wy1"), f("by1")),
        "wx2st": stack2(f("Wx2")),
        "wy2st": stack2(wy2f),
        "bx2": f("bx2").reshape(D_EMB, 1),
        "by2f": by2f.reshape(D_EMB, 1),
        "bbst": np.array(
            [[np.float32(bh[0, 0])], [np.float32(bl[0, 0])]], np.float32),
    }
    in_maps = [
        {**base, "xts": np.ascontiguousarray(xts_full[:, c * R:(c + 1) * R])}
        for c in range(N_CORES)
    ]

    if _NC is None:
        _NC = _build()
    res = run_bass_kernel_spmd(_NC, in_maps, list(range(N_CORES)), trace=TRACE)
    LAST_RESULT = res
    scores = np.concatenate(
        [res.results[c]["scores"] for c in range(N_CORES)], axis=0
    )
    return scores, np.zeros((), np.float32)
